# revision 8
# baseline (speedup 1.0000x reference)
"""Trainium2 Bass kernel for nn_CrossAttentionRouter.

Reference computation (B=2, L=4096, D=512, H=8 heads, NP=2048 queries):
    q  = LN(queries) broadcast over B            (parameter-only)
    xn = LN(x)                                   [B, L, D]
    qp = (q @ wq.T + bq) / sqrt(64)              [NP, D]  (parameter-only)
    kp = xn @ wk.T + bk                          [B, L, D]
    s_h = qp_h @ kp_h.T                          [B, H, NP, L]
    attn1 = mean_h softmax_k(s_h)                [B, NP, L]
    attn2 = softmax((log(attn1)+1e-9)/0.7)       ~ attn1^(1/0.7) normalized
    out = attn2 @ xn                             [B, NP, D] -> [B, 32, 64, D]

Device algorithm per core (8 cores, each owns 512 of the B*NP=4096 query
rows, so each core needs only its batch's x):
    phase 1: LN(x) -> xn (bf16), xbar-transpose -> xnT, kp projection
             (kp bias-add + psum eviction on the otherwise-idle ACT engine)
    phase 2, per 128-query block (qb), software-pipelined across qb:
      scores per (half-L, head) -> psum [128, 2048] (PE, 2-slot rotation
        over the whole 8-bank PSUM; the out-matmul borrows a slot)
      E_h = exp(s_h) on ACT at 2048 cols/instr with fused row-sum z_h
      w_h = 1/z_h  (DVE)
      v = sum_h w_h E_h accumulated in SBUF bf16 (v == H*attn1 up to a
        per-row scale, which cancels): DVE does tensor_scalar scale (4x
        mode) + tensor_tensor add (2x mode); Pool does fused
        scalar_tensor_tensor (E*w)+v chunks. No PSUM needed -> PSUM stays
        dedicated to the scores pipeline.
      u = exp(ln(v)/T) on ACT (4096-wide Ln then Exp with fused row-sum)
      out_row = (uT.T @ xn) * (1/rowsum(u))  (uT via xbar transpose)

    Engine schedule intent: ACT is the bottleneck (exp is ACT-only on this
    ISA) and must never stall; everything else is placed to keep it fed.
"""

import numpy as np
from contextlib import ExitStack

import ml_dtypes
import orjson

import concourse.bass as bass
import concourse.tile as tile
from concourse import mybir
from concourse.bass_utils import run_bass_kernel_spmd


def _legalize_bir(bir_bytes: bytes, max_waits: int = 1) -> bytes:
    """Split multi-semaphore waits onto standalone EventSemaphore instructions.

    This walrus build accepts at most one sync-wait command per engine
    instruction; the Tile scheduler emits several. Waits gate instruction
    *issue*, so hoisting them onto preceding same-engine EventSemaphore
    instructions is semantics-preserving.
    """
    d = orjson.loads(bir_bytes)
    ctr = 0
    for fn in d["functions"]:
        for blk in fn["blocks"]:
            out = []
            for ins in blk["instructions"]:
                si = ins.get("sync_info")
                if si:
                    w = si.get("on_wait") or []
                    if len(w) > max_waits:
                        for wi in w[:-max_waits]:
                            ctr += 1
                            out.append({
                                "debug": ins.get("debug", 0),
                                "engine": ins["engine"],
                                "ins": [],
                                "name": f"I-legw{ctr}",
                                "opcode": "EventSemaphore",
                                "outs": [],
                                "sync_info": {"on_update": [],
                                              "on_wait": [wi]},
                            })
                        si["on_wait"] = w[-max_waits:]
                out.append(ins)
            blk["instructions"] = out
    return orjson.dumps(d)


def _patch_legalize(nc: "bass.Bass") -> "bass.Bass":
    orig = nc.to_json_bytes
    nc.to_json_bytes = lambda: _legalize_bir(orig())
    return nc

F32 = mybir.dt.float32
BF16 = mybir.dt.bfloat16
NP_BF16 = ml_dtypes.bfloat16
ALU = mybir.AluOpType
AF = mybir.ActivationFunctionType

B, L, D = 2, 4096, 512
H, HD = 8, 64
NQ = 32 * 64          # 2048 queries
NCORES = 8
QSH = B * NQ // NCORES  # 512 query rows per core
TEMP = 0.7
LN_EPS = 1e-5
NDB = D // 128        # 4 partition blocks of the projected dim

# diag (head-sum) engine per chunk, index = qtr*8 + h over 32 chunks/qb:
# 'V' = DVE scale(4x)+add(2x), 'S' = Pool scale + DVE add
DIAG_ENG = (['V'] * 8              # qtr 0
            + ['V'] * 8            # qtr 1
            + ['V'] * 8            # qtr 2
            + ['S'] * 8)           # qtr 3


def _build_body(ctx: ExitStack, tc: "tile.TileContext",
                x_in, qpt_in, wkt_in, bkp_in, out_dram,
                L_=L, QSH_=QSH):
    nc = tc.nc
    NT = L_ // 128       # l-tiles
    NQB = QSH_ // 128    # query blocks
    NQTR = L_ // 1024    # 1024-wide L quarters
    NHF = L_ // 2048     # 2048-wide L halves

    const = ctx.enter_context(tc.tile_pool(name="const", bufs=1))
    persist = ctx.enter_context(tc.tile_pool(name="persist", bufs=1))
    small = ctx.enter_context(tc.tile_pool(name="small", bufs=24))

    wkt_sb = const.tile([128, NDB * 512], BF16)    # [din_local, (dchunk, dout)]
    qpt_sb = const.tile([128, NDB * QSH_], BF16)   # [dout_local, (dblk, q)]
    bkp_sb = const.tile([128, NDB], F32)
    for c in range(NDB):
        nc.gpsimd.dma_start(wkt_sb[:, c * 512:(c + 1) * 512],
                          wkt_in[c * 128:(c + 1) * 128, :])
        nc.gpsimd.dma_start(qpt_sb[:, c * QSH_:(c + 1) * QSH_],
                          qpt_in[c * 128:(c + 1) * 128, :])
    nc.gpsimd.dma_start(bkp_sb[:], bkp_in[:, :])

    xn_sb = persist.tile([128, NT * 512], BF16)    # [l_local, (ltile, d)]
    LHK = L_ // 2 if L_ >= 2048 else L_
    kpt_h = [persist.tile([128, NDB * LHK], BF16, name=f"kpt_h{i}")
             for i in range(L_ // LHK)]           # [dout_local, (dblk, l_half)]

    # ---------------- phase 1: LN(x), xnT, K projection ----------------
    with ExitStack() as p1:
        CH = min(16, NT)                   # l-tiles per x load chunk
        xstage = p1.enter_context(tc.tile_pool(name="xstage", bufs=2))
        xnt_pool = p1.enter_context(tc.tile_pool(name="xnt", bufs=1))
        kp_ps_pool = p1.enter_context(
            tc.tile_pool(name="kp_ps", bufs=4, space="PSUM"))

        xnt_sb = xnt_pool.tile([128, NDB * L_], BF16)  # [din_local, (dblk, l)]

        chunks = []
        mvall = small.tile([128, 2 * NT], F32, tag="mvall", bufs=1)
        negmu_a = small.tile([128, NT], F32, tag="negmu_a", bufs=1)
        veps_a = small.tile([128, NT], F32, tag="veps_a", bufs=1)
        sd_a = small.tile([128, NT], F32, tag="sd_a", bufs=1)
        rs_a = small.tile([128, NT], F32, tag="rs_a", bufs=1)
        r0_a = small.tile([128, NT], F32, tag="r0_a", bufs=1)
        tnr_a = small.tile([128, NT], F32, tag="tnr_a", bufs=1)
        mv2 = mvall.rearrange("p (t k) -> p t k", k=2)
        LD = min(4, CH)                    # l-tiles per load piece
        for h0 in range(0, NT, CH):
            xch = xstage.tile([128, CH * 512], F32, tag="xch")
            chunks.append(xch)
            for g0 in range(0, CH, LD):
                src = x_in[(h0 + g0) * 128:(h0 + g0 + LD) * 128, :]
                src = src.rearrange("(c p) d -> p c d", p=128)
                dst = xch[:, g0 * 512:(g0 + LD) * 512]
                nc.gpsimd.dma_start(
                    dst.rearrange("p (c d) -> p c d", c=LD)[:, :, :], src)
            for tt in range(CH):
                t = h0 + tt
                xr = xch[:, tt * 512:(tt + 1) * 512]
                st6 = small.tile([128, 6], F32, tag="st6")
                nc.vector.bn_stats(st6[:], xr[:])
                nc.vector.bn_aggr(mvall[:, 2 * t:2 * t + 2], st6[:])
            hs = slice(h0, h0 + CH)
            nc.vector.tensor_scalar(negmu_a[:, hs], mv2[:, hs, 0], -1.0,
                                    None, ALU.mult)
            nc.vector.tensor_scalar(veps_a[:, hs], mv2[:, hs, 1], LN_EPS,
                                    None, ALU.add)
            nc.scalar.activation(sd_a[:, hs], veps_a[:, hs], AF.Sqrt)
            nc.vector.reciprocal(r0_a[:, hs], sd_a[:, hs])
            nc.vector.tensor_tensor(tnr_a[:, hs], r0_a[:, hs], r0_a[:, hs],
                                    ALU.mult)
            nc.vector.tensor_tensor(tnr_a[:, hs], tnr_a[:, hs], veps_a[:, hs],
                                    ALU.mult)
            nc.vector.tensor_scalar(tnr_a[:, hs], tnr_a[:, hs], -0.5, 1.5,
                                    ALU.mult, ALU.add)
            nc.vector.tensor_tensor(rs_a[:, hs], r0_a[:, hs], tnr_a[:, hs],
                                    ALU.mult)

            for t in range(h0, h0 + CH):
                xr = chunks[t // CH][:, (t % CH) * 512:((t % CH) + 1) * 512]
                # fused normalize + bf16 cast straight into xn_sb
                nc.vector.tensor_scalar(xn_sb[:, t * 512:(t + 1) * 512],
                                        xr[:], negmu_a[:, t:t + 1],
                                        rs_a[:, t:t + 1], ALU.add, ALU.mult)
            if True:
                t = h0 + CH - 1
                # one big batched block-transpose per CH l-tiles:
                # xnT block index c = t*NDB + db holds xn[t-tile, d-block db].T
                half0 = (t + 1 - CH)
                xnt_v = xnt_sb.rearrange("p (c l) -> p c l", c=NT * NDB)
                nc.sync.dma_start_transpose(
                    xnt_v[:, half0 * NDB:(t + 1) * NDB, :],
                    xn_sb[:, half0 * 512:(t + 1) * 512])
                # K-projection for this L span, overlapped with next half's LN
                xnt_4d = xnt_sb.rearrange("p (t b l) -> p t b l", t=NT, b=NDB)
                for db in range(NDB):
                    for ls in range(half0 * 128 // 1024,
                                    (t + 1) * 128 // 1024):
                        kp_ps = kp_ps_pool.tile([128, 1024], F32, tag="kp")
                        for c in range(NDB):
                            for hf in range(2):
                                t0 = (ls * 1024 + hf * 512) // 128
                                nc.tensor.matmul(
                                    kp_ps[:, hf * 512:(hf + 1) * 512],
                                    lhsT=wkt_sb[:, c * 512 + db * 128:
                                                c * 512 + (db + 1) * 128],
                                    rhs=xnt_4d[:, t0:t0 + 4, c, :],
                                    start=(c == 0), stop=(c == NDB - 1))
                        kh, lsl = (ls * 1024) // LHK, (ls * 1024) % LHK
                        # bias-add + psum->SBUF bf16 eviction on ACT
                        nc.scalar.activation(
                            kpt_h[kh][:, db * LHK + lsl: db * LHK + lsl + 1024],
                            kp_ps[:], AF.Identity, bias=bkp_sb[:, db:db + 1])

    # ---------------- phase 2: attention per query block ----------------
    with ExitStack() as p2:
        sc_ps_pool = p2.enter_context(
            tc.tile_pool(name="sc_ps", bufs=2, space="PSUM"))
        e_pool = p2.enter_context(tc.tile_pool(name="epool", bufs=9))
        v_pool = p2.enter_context(tc.tile_pool(name="vpool", bufs=1))
        lnv_pool = p2.enter_context(tc.tile_pool(name="lnvpool", bufs=1))
        u_pool = p2.enter_context(tc.tile_pool(name="upool", bufs=1))
        ut_pool = p2.enter_context(tc.tile_pool(name="utpool", bufs=2))
        tmp_pool = p2.enter_context(tc.tile_pool(name="tmppool", bufs=2))
        ostage = p2.enter_context(tc.tile_pool(name="ostage", bufs=2))

        state = {}   # per-qb tiles for the pipelined tail

        def scores_exps(qb):
            e_tiles = [e_pool.tile([128, L_], BF16, tag="E",
                                   name=f"E_{qb}_{h}")
                       for h in range(H)]
            zp = small.tile([128, NHF * H], F32, tag="zp")
            for half in range(NHF):
                for h in range(H):
                    r0 = 64 * (h % 2)
                    s = sc_ps_pool.tile([128, 2048], F32, tag="s")
                    for hf in range(4):
                        nc.tensor.matmul(
                            s[:, hf * 512:(hf + 1) * 512],
                            lhsT=qpt_sb[r0:r0 + 64,
                                        (h // 2) * QSH_ + qb * 128:
                                        (h // 2) * QSH_ + (qb + 1) * 128],
                            rhs=kpt_h[half][r0:r0 + 64,
                                            (h // 2) * LHK + hf * 512:
                                            (h // 2) * LHK + (hf + 1) * 512],
                            start=True, stop=True, tile_position=(r0, 0))
                    nc.scalar.activation(
                        e_tiles[h][:, half * 2048:(half + 1) * 2048],
                        s[:], AF.Exp,
                        accum_out=zp[:, half * H + h:half * H + h + 1])
            state[qb] = dict(e=e_tiles, zp=zp)

        def tail_diag(qb):
            st = state[qb]
            e_tiles, zp = st["e"], st["zp"]
            z = small.tile([128, H], F32, tag="z")
            nc.vector.tensor_tensor(z[:], zp[:, 0:H], zp[:, H:2 * H], ALU.add)
            w = small.tile([128, H], F32, tag="w")
            nc.vector.reciprocal(w[:], z[:])

            v_t = v_pool.tile([128, L_], BF16, tag="v")
            for qtr in range(NQTR):
                vsl = v_t[:, qtr * 1024:(qtr + 1) * 1024]
                for h in range(H):
                    esl = e_tiles[h][:, qtr * 1024:(qtr + 1) * 1024]
                    wv = w[:, h:h + 1]
                    eng = DIAG_ENG[qtr * 8 + h]
                    if h == 0:
                        # first head initialises v
                        if eng == 'V':
                            nc.vector.tensor_scalar(vsl, esl, wv, None,
                                                    ALU.mult)
                        else:
                            nc.gpsimd.tensor_scalar(vsl, esl, wv, None,
                                                    ALU.mult)
                    else:
                        tmp = tmp_pool.tile([128, 1024], BF16, tag="tmp")
                        if eng == 'V':
                            nc.vector.tensor_scalar(tmp[:], esl, wv, None,
                                                    ALU.mult)
                        else:
                            nc.gpsimd.tensor_scalar(tmp[:], esl, wv, None,
                                                    ALU.mult)
                        nc.vector.tensor_tensor(vsl, vsl, tmp[:], ALU.add)
            st["v"] = v_t

        def tail_u(qb):
            st = state[qb]
            v_t = st["v"]
            # u = exp(ln(v)/T) with fused row-sum; u overwrites v's tile
            lnv = lnv_pool.tile([128, L_], BF16, tag="lnv")
            nc.scalar.activation(lnv[:], v_t[:], AF.Ln)
            u_t = u_pool.tile([128, L_], BF16, tag="u")
            us = small.tile([128, 1], F32, tag="us")
            nc.scalar.activation(u_t[:], lnv[:], AF.Exp, scale=1.0 / TEMP,
                                 accum_out=us[:])
            rus = small.tile([128, 1], F32, tag="rus")
            nc.vector.reciprocal(rus[:], us[:])
            st["rus"] = rus

            # transpose u -> uT in xbar chunks
            ut_ts = []
            for lo in range(0, L_, 2048):
                ut_t = ut_pool.tile([128, 2048], BF16, tag="uT")
                ut_v = ut_t.rearrange("p (c l) -> p c l", c=16)
                nc.sync.dma_start_transpose(ut_v[:, :, :],
                                            u_t[:, lo:lo + 2048])
                ut_ts.append(ut_t)
            st["ut"] = ut_ts

        def tail_pe(qb):
            st = state[qb]
            ut_ts, rus = st["ut"], st["rus"]
            out_ps_t = sc_ps_pool.tile([128, 2048], F32, tag="s",
                                       name=f"ops_{qb}")
            out_ps = out_ps_t[:, 0:512]
            for c in range(NT):
                ut_t = ut_ts[c // 16]
                nc.tensor.matmul(out_ps,
                                 lhsT=ut_t[:, (c % 16) * 128:
                                           (c % 16 + 1) * 128],
                                 rhs=xn_sb[:, c * 512:(c + 1) * 512],
                                 start=(c == 0), stop=(c == NT - 1))
            outf = ostage.tile([128, 512], F32, tag="outf")
            nc.vector.tensor_scalar(outf[:], out_ps, rus[:], None,
                                    ALU.mult)
            nc.gpsimd.dma_start(out_dram[qb * 128:(qb + 1) * 128, :], outf[:])
            del state[qb]

        for qb in range(NQB):
            if qb >= 1:
                tail_diag(qb - 1)
            scores_exps(qb)
            if qb >= 1:
                tail_u(qb - 1)
                tail_pe(qb - 1)
        tail_diag(NQB - 1)
        tail_u(NQB - 1)
        tail_pe(NQB - 1)


def build_nc(L_=L, QSH_=QSH):
    nc = bass.Bass()
    x_in = nc.declare_dram_parameter("x_b", [L_, D], F32, isOutput=False)
    qpt_in = nc.declare_dram_parameter("qpt", [D, QSH_], BF16, isOutput=False)
    wkt_in = nc.declare_dram_parameter("wkt", [D, D], BF16, isOutput=False)
    bkp_in = nc.declare_dram_parameter("bkp", [128, NDB], F32, isOutput=False)
    out_dram = nc.declare_dram_parameter("out", [QSH_, D], F32, isOutput=True)
    with ExitStack() as ctx:
        tc = ctx.enter_context(tile.TileContext(nc))
        _build_body(ctx, tc, x_in, qpt_in, wkt_in, bkp_in, out_dram,
                    L_=L_, QSH_=QSH_)
    return _patch_legalize(nc)


def host_prep(x, queries, wq, wk, bq, bk, gamma_q, beta_q, gamma_x, beta_x,
              L_=L, QSH_=QSH, ncores=NCORES):
    """Parameter-only host prep + per-core input maps."""
    x = np.asarray(x, np.float32)
    queries = np.asarray(queries, np.float32)
    wq = np.asarray(wq, np.float32)
    wk = np.asarray(wk, np.float32)
    bq = np.asarray(bq, np.float32)
    bk = np.asarray(bk, np.float32)
    gamma_q = np.asarray(gamma_q, np.float32)
    beta_q = np.asarray(beta_q, np.float32)
    gamma_x = np.asarray(gamma_x, np.float32)
    beta_x = np.asarray(beta_x, np.float32)

    # fold LN affines into the projections (exact):
    #   kp = (LN0(x)*gx + bx) @ wk.T + bk = LN0(x) @ (wk*gx).T + (wk@bx + bk)
    wq_f = wq * gamma_q[None, :]
    bq_f = wq @ beta_q + bq
    wk_f = wk * gamma_x[None, :]
    bk_f = wk @ beta_x + bk

    # parameter-only query path
    qflat = queries.reshape(NQ, D)
    mu = qflat.mean(-1, keepdims=True)
    var = ((qflat - mu) ** 2).mean(-1, keepdims=True)
    qn = (qflat - mu) / np.sqrt(var + LN_EPS)
    qp = (qn @ wq_f.T + bq_f) * np.float32(1.0 / np.sqrt(HD))  # [NQ, D]

    nqb_total = B * NQ // QSH_  # shards across batches*queries
    per_batch = nqb_total // B
    in_maps = []
    wkt_np = np.ascontiguousarray(wk_f.T).astype(NP_BF16)
    bkp_np = np.ascontiguousarray(bk_f.reshape(NDB, 128).T).astype(np.float32)
    for c in range(ncores):
        b = c // per_batch
        q0 = (c % per_batch) * QSH_
        in_maps.append(dict(
            x_b=np.ascontiguousarray(x[b, :L_, :]),
            qpt=np.ascontiguousarray(qp[q0:q0 + QSH_].T).astype(NP_BF16),
            wkt=wkt_np,
            bkp=bkp_np,
        ))
    return in_maps, (gamma_x, beta_x)


_NC_CACHE = {}


def _get_nc(L_=L, QSH_=QSH):
    key = (L_, QSH_)
    if key not in _NC_CACHE:
        _NC_CACHE[key] = build_nc(L_, QSH_)
    return _NC_CACHE[key]


def run_sharded(inputs, trace=False):
    in_maps, (gamma_x, beta_x) = host_prep(**inputs)
    nc = _get_nc()
    res = run_bass_kernel_spmd(nc, in_maps, list(range(NCORES)), trace=trace)
    outs = [res.results[c]["out"] for c in range(NCORES)]
    out = np.concatenate(outs, axis=0).reshape(B, NQ, D)
    if not (np.allclose(gamma_x, 1.0) and np.allclose(beta_x, 0.0)):
        out = out * gamma_x[None, None, :] + beta_x[None, None, :]
    return out.reshape(B, 32, 64, D).astype(np.float32), res


def kernel(**inputs):
    out, _ = run_sharded(inputs, trace=False)
    return out
